# revision 14
# baseline (speedup 1.0000x reference)
"""Trainium2 Bass kernel for the unit-commitment custom loss (v3).

Strategy (8 NeuronCores, SPMD single program):
  - Min-up/down-time violations via a per-row prefix-sum (one DVE scan per
    tile) plus clamped shifted-gather DMAs (SBUF->SBUF), replacing the
    O(max(U)+max(D)) lag-correlation passes:
        viol_up(row)  = sum_t sw_on[t] *(U - cs[t+U] + cs[t]) + corner corr
        viol_dn(row)  = sum_t sw_off[t]*(cs[t+D] - cs[t])     + corner corr
    The gather shift is the per-generator U (resp. D), host-known, so rows
    are grouped by (U,D) class and gathered with one strided DMA per class
    run. SPMD requires identical run boundaries on all cores: each class's
    count is truncated to a multiple of 8; the few leftover generators
    (c%8 per class, ~220 total) have their violation terms computed on the
    host (<0.3% of the element work). BCE/seg/pg for ALL generators stays
    on device.
  - All elementwise work is fp16 (DVE 2x mode for tensor_tensor; counts
    <= 2048 are exact in fp16). Products+row-reductions use
    scalar_tensor_tensor accum (1x) - only 2 per tile now.
  - seg_prod is staged fp8e4m3 as [B*T, G*K] and reduced on the idle
    tensor engine with ones-vector matmuls, as are profiled_generation
    and charge/discharge rate row sums.
  - BCE terms use the binary-target select trick: q = 0.5+(2t-1)(p-0.5),
    with the affine preps on the scalar engine, the product on DVE (2x),
    and the Ln+accumulate on the scalar engine.
"""

import numpy as np
import ml_dtypes

B, G, T, K, P, S = 16, 4000, 96, 4, 500, 200
M = 8            # cores
GC = G // M      # 500 generators per core (BCE tiles)
BS = B // M      # 2 scenarios per core (B-sharded tensors)
GT = 4           # tile slots per core
BT = B * T       # 1536
NCOL = 48
VIOLATIONS_PENALTY = 1000.0
POWER_BALANCE_PENALTY = 5000.0

# out_g column map (violation rows; accumulated with nsw = -sw sign)
C_SWON = 0       # sum nsw_on
C_GUP = 1        # sum nsw_on * (gup - cs)
C_GDN = 2        # sum nsw_off * (gdn - cs)
C_SWT0 = 16      # cols 16..22 : i=0..6 -> tau=7-i : sum_b nsw_on[b, T-tau]
C_XON0 = 23      # cols 23..29 : i -> tau=7-i : sum_b nsw_on[b,T-tau]*cs_raw[T-tau]
C_XOFF0 = 30     # cols 30..36
C_PFB0 = 37      # cols 37..45 : P_raw(r), r=0..8 (host: P(r)-P(0))

_NC_CACHE = {}


def _build_nc(plan_key):
    """plan_key = (tile_sizes, on_runs, off_runs) as nested tuples."""
    import concourse.bacc as bacc
    import concourse.tile as tile
    import concourse.mybir as mybir

    tile_sizes, on_runs, off_runs = plan_key
    rows_pc = sum(tile_sizes)

    f32 = mybir.dt.float32
    f16 = mybir.dt.float16
    f8 = mybir.dt.float8e4
    alu = mybir.AluOpType
    AX = mybir.AxisListType
    LN = mybir.ActivationFunctionType.Ln
    CP = mybir.ActivationFunctionType.Copy

    nc = bacc.Bacc("TRN2", target_bir_lowering=False, debug=False, num_devices=M)

    # const AP for the Ln(v + 0.5) bias used by the BCE select trick
    _half = nc.alloc_sbuf_tensor("const-half", [128, 1], f32)
    nc.gpsimd.memset(_half.ap(), 0.5)
    nc.const_aps.aps[(f32, 0.5)] = _half.ap()
    nc.all_engine_barrier()

    sE = nc.dram_tensor("sE", [rows_pc, BT], f16, kind="ExternalInput").ap()
    pE = nc.dram_tensor("pE", [rows_pc, BT], f16, kind="ExternalInput").ap()
    pp = nc.dram_tensor("pp", [GC, BT], f16, kind="ExternalInput").ap()
    tt = nc.dram_tensor("tt", [GC, BT], f16, kind="ExternalInput").ap()
    sp8 = nc.dram_tensor("sp8", [BT, GC * K], f8, kind="ExternalInput").ap()
    pg16 = nc.dram_tensor("pg16", [T, BS * P], f16, kind="ExternalInput").ap()
    st6 = nc.dram_tensor("st6", [T, 6 * S * BS + BS], f16, kind="ExternalInput").ap()
    ones16 = nc.dram_tensor("ones16", [128, 2], f16, kind="ExternalInput").ap()
    ones8 = nc.dram_tensor("ones8", [128, 2], f8, kind="ExternalInput").ap()

    out_g = nc.dram_tensor("out_g", [max(rows_pc, 1), NCOL], f32,
                           kind="ExternalOutput").ap()
    out_b = nc.dram_tensor("out_b", [GC, 2], f32, kind="ExternalOutput").ap()
    out_s = nc.dram_tensor("out_s", [T, 8], f32, kind="ExternalOutput").ap()
    out_pe = nc.dram_tensor("out_pe", [1, 3300], f32, kind="ExternalOutput").ap()

    # round-robin engines for gather DMA issue (keep sync queue light)
    def dma_engines():
        while True:
            yield nc.sync
            yield nc.scalar
    dmae = dma_engines()

    with tile.TileContext(nc) as tc:
        with (
            tc.tile_pool(name="inp", bufs=2) as inp,
            tc.tile_pool(name="work", bufs=2) as work,
            tc.tile_pool(name="colp", bufs=2) as colp,
            tc.tile_pool(name="cst", bufs=1) as cst,
            tc.tile_pool(name="ps", bufs=1, space="PSUM") as psp,
        ):
            o16 = cst.tile([128, 2], f16, tag="o16")
            nc.sync.dma_start(o16[:], ones16[:, :])
            o8 = cst.tile([128, 2], f8, tag="o8")
            nc.sync.dma_start(o8[:], ones8[:, :])

            # ---- seg_prod reduction on the tensor engine ----
            ps_seg = [psp.tile([1, 500], f32, tag=f"seg{q}", name=f"ps_seg{q}")
                      for q in range(4)]
            for ch in range(12):
                t8 = inp.tile([128, GC * K], f8, tag="sp")
                nc.sync.dma_start(t8[:], sp8[128 * ch:128 * (ch + 1), :])
                for q in range(4):
                    nc.tensor.matmul(
                        ps_seg[q][:], o8[:, 0:1], t8[:, 500 * q:500 * (q + 1)],
                        start=(ch == 0), stop=(ch == 11))

            # ---- profiled_generation row sums ----
            ps_pg = psp.tile([1, P], f32, tag="pg")
            pgt = inp.tile([T, BS * P], f16, tag="pg")
            nc.sync.dma_start(pgt[:], pg16[:, :])
            for b in range(2):
                nc.tensor.matmul(
                    ps_pg[:], o16[0:96, 0:1], pgt[:, P * b:P * (b + 1)],
                    start=(b == 0), stop=(b == 1))

            # ---- storage block ----
            SB = S * BS  # 400
            stt = inp.tile([T, 6 * SB + BS], f16, tag="st")
            nc.sync.dma_start(stt[:], st6[:, :])
            ps_cr = psp.tile([1, SB], f32, tag="cr")
            nc.tensor.matmul(ps_cr[:], o16[0:96, 0:1], stt[:, 4 * SB:5 * SB],
                             start=True, stop=True)
            ps_dr = psp.tile([1, SB], f32, tag="dr")
            nc.tensor.matmul(ps_dr[:], o16[0:96, 0:1], stt[:, 5 * SB:6 * SB],
                             start=True, stop=True)

            scols = colp.tile([T, 8], f32, tag="scols")
            nc.vector.memset(scols[:], 0.0)
            su = work.tile([T, SB], f16, tag="su")
            sv = work.tile([T, SB], f16, tag="sv")
            sq = work.tile([T, SB], f16, tag="sq")
            for i in range(2):
                pr = stt[:, (2 * i) * SB:(2 * i + 1) * SB]
                tg = stt[:, (2 * i + 1) * SB:(2 * i + 2) * SB]
                nc.scalar.activation(su[:], tg, CP, bias=-1.0, scale=2.0)
                nc.scalar.activation(sq[:], pr, CP, bias=-0.5, scale=1.0)
                nc.vector.tensor_tensor(out=sv[:], in0=sq[:], in1=su[:],
                                        op=alu.mult)
                nc.scalar.activation(sq[:], sv[:], LN, bias=0.5, scale=1.0,
                                     accum_out=scols[:, i:i + 1])
            nc.vector.tensor_reduce(
                scols[:, 2:3], stt[:, 6 * SB:6 * SB + BS], axis=AX.X, op=alu.add)
            nc.sync.dma_start(out_s[:, :], scols[:])

            # ---- assemble PE results ----
            peout = colp.tile([1, 3300], f32, tag="peout")
            for q in range(4):
                nc.scalar.activation(
                    peout[:, 500 * q:500 * (q + 1)], ps_seg[q][:], CP)
            nc.scalar.activation(peout[:, 2000:2500], ps_pg[:], CP)
            nc.scalar.activation(peout[:, 2500:2900], ps_cr[:], CP)
            nc.scalar.activation(peout[:, 2900:3300], ps_dr[:], CP)
            nc.sync.dma_start(out_pe[:, :], peout[:])

            # ---- BCE tiles (all 500 generators, unpermuted order) ----
            GB = GC // GT  # 125
            for it in range(GT):
                r0 = it * GB
                p_t = inp.tile([GB, BT], f16, tag="pp")
                nc.sync.dma_start(p_t[:], pp[r0:r0 + GB, :])
                t_t = inp.tile([GB, BT], f16, tag="tt")
                nc.sync.dma_start(t_t[:], tt[r0:r0 + GB, :])
                bcol = colp.tile([GB, 2], f32, tag="bcol")
                nc.vector.memset(bcol[:], 0.0)
                ub = work.tile([GB, BT], f16, tag="ub")
                p5 = work.tile([GB, BT], f16, tag="p5")
                nc.scalar.activation(ub[:], t_t[:], CP, bias=-1.0, scale=2.0)
                nc.scalar.activation(p5[:], p_t[:], CP, bias=-0.5, scale=1.0)
                vv = work.tile([GB, BT], f16, tag="vv")
                nc.vector.tensor_tensor(out=vv[:], in0=p5[:], in1=ub[:],
                                        op=alu.mult)
                nc.scalar.activation(ub[:], vv[:], LN, bias=0.5, scale=1.0,
                                     accum_out=bcol[:, 0:1])
                nc.sync.dma_start(out_b[r0:r0 + GB, :], bcol[:])

            # ---- violation tiles (class-aligned rows) ----
            r_base = 0
            for it in range(GT):
                GR = tile_sizes[it]
                r1g = r_base + GR
                se = inp.tile([128, BT], f16, tag="se")
                nc.sync.dma_start(se[0:GR, :], sE[r_base:r1g, :])
                pe = inp.tile([128, BT], f16, tag="pe")
                nc.sync.dma_start(pe[0:GR, :], pE[r_base:r1g, :])

                cols = colp.tile([128, NCOL], f32, tag="cols")
                nc.vector.memset(cols[:], 0.0)

                sev = se[:].rearrange("g (b t) -> g b t", b=B)

                non = work.tile([128, BT], f16, tag="non")
                nof = work.tile([128, BT], f16, tag="nof")
                # nsw_on = (prev - 1) * s ; nsw_off = (s - 1) * prev
                nc.vector.scalar_tensor_tensor(
                    out=non[0:GR], in0=pe[0:GR], scalar=1.0, in1=se[0:GR],
                    op0=alu.subtract, op1=alu.mult,
                    accum_out=cols[0:GR, C_SWON:C_SWON + 1])
                nc.vector.scalar_tensor_tensor(
                    out=nof[0:GR], in0=se[0:GR], scalar=1.0, in1=pe[0:GR],
                    op0=alu.subtract, op1=alu.mult)

                # prefix scan: csx[g, 1+i] = running sum of s over (b t)
                csx = work.tile([128, BT + 8], f16, tag="csx")
                nc.vector.memset(csx[0:GR, 0:1], 0.0)
                nc.vector.tensor_tensor_scan(
                    out=csx[0:GR, 1:BT + 1], data0=se[0:GR], data1=se[0:GR],
                    initial=0.0, op0=alu.add, op1=alu.bypass)

                # gathers: per class run (r0,r1,shift), clamped at seg end;
                # the t > T-shift tail is left ZERO (memset) and corrected
                # on the host via the XON/XOFF columns.
                gup = work.tile([128, BT], f16, tag="gup")
                gdn = work.tile([128, BT], f16, tag="gdn")
                nc.vector.memset(gup[0:GR], 0.0)
                nc.vector.memset(gdn[0:GR], 0.0)
                for dst, runs in ((gup, on_runs[it]), (gdn, off_runs[it])):
                    for (rr0, rr1, u) in runs:
                        gv = dst[rr0:rr1].rearrange("g (b t) -> g b t", b=B)
                        src = csx[rr0:rr1, u:u + BT].rearrange(
                            "g (b t) -> g b t", b=B)[:, :, 0:97 - u]
                        next(dmae).dma_start(gv[:, :, 0:97 - u], src)

                # dU = gup - cs ; dD = gdn - cs   (exact small ints in fp16)
                dU = work.tile([128, BT], f16, tag="dU")
                nc.vector.tensor_sub(dU[0:GR], gup[0:GR], csx[0:GR, 0:BT])
                nc.vector.scalar_tensor_tensor(
                    out=gup[0:GR], in0=non[0:GR], scalar=1.0, in1=dU[0:GR],
                    op0=alu.mult, op1=alu.mult,
                    accum_out=cols[0:GR, C_GUP:C_GUP + 1])
                nc.vector.tensor_sub(dU[0:GR], gdn[0:GR], csx[0:GR, 0:BT])
                nc.vector.scalar_tensor_tensor(
                    out=gdn[0:GR], in0=nof[0:GR], scalar=1.0, in1=dU[0:GR],
                    op0=alu.mult, op1=alu.mult,
                    accum_out=cols[0:GR, C_GDN:C_GDN + 1])

                # corner columns: XON(tau=7-i) = sum_b nsw_on[b,89+i]*cs_raw[b,89+i]
                nv = non[:].rearrange("g (b t) -> g b t", b=B)
                fv = nof[:].rearrange("g (b t) -> g b t", b=B)
                non7 = nv[0:GR, :, T - 7:T]
                nof7 = fv[0:GR, :, T - 7:T]
                csx7 = csx[0:GR, 0:BT].rearrange(
                    "g (b t) -> g b t", b=B)[:, :, T - 7:T]
                prd = work.tile([128, B * 7], f16, tag="prd")
                pv = prd[:].rearrange("g (b i) -> g b i", b=B)
                pvr = prd[:].rearrange("g (b i) -> g i b", b=B)

                nc.vector.tensor_mul(pv[0:GR], non7, csx7)
                nc.vector.tensor_reduce(
                    cols[0:GR, C_XON0:C_XON0 + 7], pvr[0:GR],
                    axis=AX.X, op=alu.add)
                nc.vector.tensor_mul(pv[0:GR], nof7, csx7)
                nc.vector.tensor_reduce(
                    cols[0:GR, C_XOFF0:C_XOFF0 + 7], pvr[0:GR],
                    axis=AX.X, op=alu.add)

                # SWT: sum_b nsw_on[b, 89+i]
                non7r = non[:].rearrange("g (b t) -> g t b", b=B)[0:GR, T - 7:T, :]
                nc.vector.tensor_reduce(
                    cols[0:GR, C_SWT0:C_SWT0 + 7], non7r, axis=AX.X, op=alu.add)

                # prefix columns from csx: P_raw(r) = sum_b csx[96b + r]
                nc.vector.tensor_reduce(
                    cols[0:GR, C_PFB0:C_PFB0 + 9],
                    csx[0:GR, 0:BT].rearrange(
                        "g (b r) -> g r b", b=B)[:, 0:9, :],
                    axis=AX.X, op=alu.add)

                nc.sync.dma_start(out_g[r_base:r1g, :], cols[0:GR])
                r_base = r1g

    nc.compile()
    return nc


def _get_nc(plan_key):
    if plan_key not in _NC_CACHE:
        _NC_CACHE[plan_key] = _build_nc(plan_key)
    return _NC_CACHE[plan_key]


def _plan(U, D):
    """Class-aligned row plan.

    Returns (plan_key, rows[core] -> list of g ids (len rows_pc),
    host_g -> leftover g ids).
    """
    idx = {}
    for g in range(G):
        idx.setdefault((int(U[g]), int(D[g])), []).append(g)
    # snake order: u desc; d direction alternates per u-block so adjacent
    # blocks share d at the boundary (merges OFF gather runs)
    u_vals = sorted({u for (u, _) in idx.keys()}, reverse=True)
    classes = []
    for k, u in enumerate(u_vals):
        ds = sorted({d for (uu, d) in idx.keys() if uu == u}, reverse=(k % 2 == 0))
        classes.extend((u, d) for d in ds)
    rows = [[] for _ in range(M)]
    host_g = []
    spans = []  # (row0, row1, u, d) in per-core row space
    r = 0
    for ud in classes:
        ids = idx[ud]
        n = len(ids) // M
        if n:
            for c in range(M):
                rows[c].extend(ids[c * n:(c + 1) * n])
            spans.append((r, r + n, ud[0], ud[1]))
            r += n
        host_g.extend(ids[M * n:])
    rows_pc = r
    # tile sizes: as equal as possible, <= 128
    nt = GT
    base = rows_pc // nt
    sizes = [base + (1 if i < rows_pc % nt else 0) for i in range(nt)]
    assert max(sizes) <= 128, sizes
    # split spans at tile boundaries; build per-tile run lists
    bounds = np.cumsum([0] + sizes)
    on_runs = [[] for _ in range(nt)]
    off_runs = [[] for _ in range(nt)]
    for (s0, s1, u, d) in spans:
        for ti in range(nt):
            lo = max(s0, int(bounds[ti]))
            hi = min(s1, int(bounds[ti + 1]))
            if lo < hi:
                t0 = int(bounds[ti])
                off_runs[ti].append((lo - t0, hi - t0, d))
                # merge ON runs with same shift
                if on_runs[ti] and on_runs[ti][-1][1] == lo - t0 \
                        and on_runs[ti][-1][2] == u:
                    on_runs[ti][-1] = (on_runs[ti][-1][0], hi - t0, u)
                else:
                    on_runs[ti].append((lo - t0, hi - t0, u))
    # merge OFF runs with same consecutive shift too
    for ti in range(nt):
        merged = []
        for run in off_runs[ti]:
            if merged and merged[-1][1] == run[0] and merged[-1][2] == run[2]:
                merged[-1] = (merged[-1][0], run[1], run[2])
            else:
                merged.append(run)
        off_runs[ti] = merged
        on_runs[ti] = [tuple(x) for x in on_runs[ti]]
    plan_key = (tuple(sizes),
                tuple(tuple(rl) for rl in on_runs),
                tuple(tuple(rl) for rl in off_runs))
    return plan_key, rows, host_g


def _f16(a):
    return np.ascontiguousarray(a, dtype=np.float16)


def _prep_in_maps(inputs, rows):
    ic = np.asarray(inputs["initial_commitment"], dtype=np.float32)      # (B,G)
    s_full = np.asarray(inputs["thermal_on_rounded"], dtype=np.float32)  # (B,G,T)
    p_full = np.asarray(inputs["thermal_on"], dtype=np.float32)
    t_full = np.asarray(inputs["tgt_thermal_commitment"], dtype=np.float32)
    sp_full = np.asarray(inputs["seg_prod"], dtype=np.float32)           # (B,G,T,K)
    pg_full = np.asarray(inputs["profiled_generation"], dtype=np.float32)
    chp = np.asarray(inputs["is_charging"], dtype=np.float32)            # (B,S,T)
    cht = np.asarray(inputs["tgt_is_charging"], dtype=np.float32)
    dsp = np.asarray(inputs["is_discharging"], dtype=np.float32)
    dst = np.asarray(inputs["tgt_is_discharging"], dtype=np.float32)
    cr = np.asarray(inputs["charge_rate"], dtype=np.float32)
    dr = np.asarray(inputs["discharge_rate"], dtype=np.float32)
    curt = np.asarray(inputs["curtailment"], dtype=np.float32)           # (B,T)

    ones16 = np.ones((128, 2), dtype=np.float16)
    ones8 = np.ones((128, 2), dtype=ml_dtypes.float8_e4m3)

    in_maps = []
    for c in range(M):
        gids = np.asarray(rows[c], dtype=np.int64)
        s = s_full[:, gids, :].transpose(1, 0, 2)            # (rows,B,T)
        n = len(gids)
        pEc = np.empty((n, B, T), dtype=np.float16)
        pEc[:, :, 0] = ic[:, gids].T
        pEc[:, :, 1:] = s[:, :, :T - 1]

        gsl = slice(GC * c, GC * (c + 1))
        bsl = slice(BS * c, BS * (c + 1))
        sp8c = np.ascontiguousarray(
            sp_full[:, gsl].transpose(0, 2, 1, 3).reshape(BT, GC * K)
        ).astype(ml_dtypes.float8_e4m3)
        # (BS,P,T) -> (T, BS*P), col = b*P + p
        pgc = _f16(pg_full[bsl].transpose(2, 0, 1).reshape(T, BS * P))

        def sb(x):  # (BS,S,T) -> (T, S*BS) with col = s*BS + b
            return x[bsl].transpose(2, 1, 0).reshape(T, S * BS)

        st6c = np.concatenate(
            [sb(chp), sb(cht), sb(dsp), sb(dst), sb(cr), sb(dr),
             curt[bsl].T], axis=1)

        in_maps.append({
            "sE": _f16(s).reshape(n, BT),
            "pE": pEc.reshape(n, BT),
            "pp": _f16(p_full[:, gsl].transpose(1, 0, 2)).reshape(GC, BT),
            "tt": _f16(t_full[:, gsl].transpose(1, 0, 2)).reshape(GC, BT),
            "sp8": sp8c,
            "pg16": pgc,
            "st6": _f16(st6c),
            "ones16": ones16,
            "ones8": ones8,
        })
    return in_maps


def kernel(**inputs):
    from concourse.bass_utils import run_bass_kernel_spmd

    U_all = np.maximum(np.asarray(inputs["min_uptimes"]).astype(np.int64), 0)
    D_all = np.maximum(np.asarray(inputs["min_downtimes"]).astype(np.int64), 0)
    plan_key, rows, host_g = _plan(U_all, D_all)
    nc = _get_nc(plan_key)
    in_maps = _prep_in_maps(inputs, rows)
    res = run_bass_kernel_spmd(nc, in_maps, core_ids=list(range(M)))
    return _combine(res.results, inputs, rows, host_g)


def _host_viol(inputs, gids):
    """Reference-exact violations + switch-on counts for leftover g ids."""
    if len(gids) == 0:
        return 0.0, np.zeros(0)
    gids = np.asarray(gids, dtype=np.int64)
    s = np.asarray(inputs["thermal_on_rounded"], dtype=np.float64)[:, gids, :]
    ic = np.asarray(inputs["initial_commitment"], dtype=np.float64)[:, gids]
    U = np.maximum(np.asarray(inputs["min_uptimes"]).astype(np.int64)[gids], 0)
    D = np.maximum(np.asarray(inputs["min_downtimes"]).astype(np.int64)[gids], 0)
    stat = np.asarray(inputs["initial_status"]).astype(np.int64)[gids]

    prev = np.concatenate([ic[:, :, None], s[:, :, :-1]], axis=2)
    sw_on = (1.0 - prev) * s
    sw_off = prev * (1.0 - s)

    def mtv(sw, series, early_series, W, remaining):
        Bn, Gn, Tn = series.shape
        t = np.arange(Tn)
        early = (t[None, None, :] < remaining[None, :, None]) * early_series
        cs = np.concatenate([np.zeros((Bn, Gn, 1)), np.cumsum(series, axis=-1)],
                            axis=-1)
        end = t[None, :] + W[:, None]
        idx = np.clip(end, 0, Tn)
        wsum = np.take_along_axis(
            cs, np.broadcast_to(idx[None], (Bn, Gn, Tn)), axis=-1) - cs[:, :, :Tn]
        valid = ((end <= Tn) & (W[:, None] > 0))[None]
        viol = np.maximum(W[:, None][None] - wsum, 0.0) * valid * sw
        return early + viol

    rem_up = np.maximum(U - np.maximum(stat, 0), 0)
    rem_dn = np.maximum(D - np.maximum(-stat, 0), 0)
    up_v = mtv(sw_on, s, 1.0 - s, U, rem_up)
    dn_v = mtv(sw_off, 1.0 - s, s, D, rem_dn)
    swon_per_g = sw_on.sum(axis=(0, 2))
    return float(up_v.sum() + dn_v.sum()), swon_per_g


def _combine(results, inputs, rows, host_g):
    U_all = np.asarray(inputs["min_uptimes"]).astype(np.int64)
    D_all = np.asarray(inputs["min_downtimes"]).astype(np.int64)
    stat_all = np.asarray(inputs["initial_status"]).astype(np.int64)
    suc_all = np.asarray(inputs["start_up_costs"], dtype=np.float64)
    segc_all = np.asarray(inputs["segment_cost"], dtype=np.float64)[:, 0, :]
    puc = np.asarray(inputs["profiled_units_cost"], dtype=np.float64)
    ccost = np.asarray(inputs["charge_costs"], dtype=np.float64)
    dcost = np.asarray(inputs["discharge_costs"], dtype=np.float64)

    tt_i = 7 - np.arange(7)[None, :]       # column i -> tau = 7-i

    viol = 0.0
    ed = 0.0
    bce_th = 0.0
    bce_ch = 0.0
    bce_ds = 0.0
    curt_sum = 0.0

    for c in range(M):
        o = np.asarray(results[c]["out_g"], dtype=np.float64)
        ob = np.asarray(results[c]["out_b"], dtype=np.float64)
        ope = np.asarray(results[c]["out_pe"], dtype=np.float64)[0]
        osr = np.asarray(results[c]["out_s"], dtype=np.float64)

        gids = np.asarray(rows[c], dtype=np.int64)
        n = len(gids)
        o = o[:n]
        U = U_all[gids]
        D = D_all[gids]
        stat = stat_all[gids]

        SWON = -o[:, C_SWON]
        A_on = o[:, C_GUP] - U * o[:, C_SWON]   # = U*SWON - sum sw*dU
        A_off = -o[:, C_GDN]
        SWT = -o[:, C_SWT0:C_SWT0 + 7]
        XON = -o[:, C_XON0:C_XON0 + 7]
        XOFF = -o[:, C_XOFF0:C_XOFF0 + 7]
        Praw = o[:, C_PFB0:C_PFB0 + 9]
        Pf = Praw - Praw[:, 0:1]

        mU = (tt_i < U[:, None])
        up = A_on.sum()
        up -= (U[:, None] * SWT * mU).sum()
        up -= (XON * mU).sum()
        dn = A_off.sum()
        dn += (XOFF * (tt_i < D[:, None])).sum()
        rem_up = np.maximum(U - np.maximum(stat, 0), 0)
        rem_dn = np.maximum(D - np.maximum(-stat, 0), 0)
        g_idx = np.arange(n)
        early = (B * rem_up - Pf[g_idx, rem_up]).sum() + Pf[g_idx, rem_dn].sum()
        viol += up + dn + early

        ed += (suc_all[gids] * SWON).sum()
        bce_th += ob[:, 0].sum()

        gsl = slice(GC * c, GC * (c + 1))
        ed += (segc_all[gsl].reshape(-1) * ope[0:2000]).sum()
        ed += (puc * ope[2000:2500]).sum()
        ed += (ccost * ope[2500:2900].reshape(S, BS).sum(axis=1)).sum()
        ed += (dcost * ope[2900:3300].reshape(S, BS).sum(axis=1)).sum()

        bce_ch += osr[:, 0].sum()
        bce_ds += osr[:, 1].sum()
        curt_sum += osr[:, 2].sum()

    # leftover generators fully on host (violations + startup term)
    hviol, hswon = _host_viol(inputs, host_g)
    viol += hviol
    if len(host_g):
        ed += (suc_all[np.asarray(host_g, dtype=np.int64)] * hswon).sum()

    n_th = float(B * G * T)
    n_s = float(B * S * T)
    sup = -(bce_th / n_th) - (bce_ch / n_s) - (bce_ds / n_s)
    total = (ed + POWER_BALANCE_PENALTY * curt_sum + sup
             + VIOLATIONS_PENALTY * viol)
    return np.float32(total)


# revision 15
# speedup vs baseline: 1.1034x; 1.1034x over previous
"""Trainium2 Bass kernel for the unit-commitment custom loss (v3).

Strategy (8 NeuronCores, SPMD single program):
  - Min-up/down-time violations via a per-row prefix-sum (one DVE scan per
    tile) plus clamped shifted-gather DMAs (SBUF->SBUF), replacing the
    O(max(U)+max(D)) lag-correlation passes:
        viol_up(row)  = sum_t sw_on[t] *(U - cs[t+U] + cs[t]) + corner corr
        viol_dn(row)  = sum_t sw_off[t]*(cs[t+D] - cs[t])     + corner corr
    The gather shift is the per-generator U (resp. D), host-known, so rows
    are grouped by (U,D) class and gathered with one strided DMA per class
    run. SPMD requires identical run boundaries on all cores: each class's
    count is truncated to a multiple of 8; the few leftover generators
    (c%8 per class, ~220 total) have their violation terms computed on the
    host (<0.3% of the element work). BCE/seg/pg for ALL generators stays
    on device.
  - All elementwise work is fp16 (DVE 2x mode for tensor_tensor; counts
    <= 2048 are exact in fp16). Products+row-reductions use
    scalar_tensor_tensor accum (1x) - only 2 per tile now.
  - seg_prod is staged fp8e4m3 as [B*T, G*K] and reduced on the idle
    tensor engine with ones-vector matmuls, as are profiled_generation
    and charge/discharge rate row sums.
  - BCE terms use the binary-target select trick: q = 0.5+(2t-1)(p-0.5),
    with the affine preps on the scalar engine, the product on DVE (2x),
    and the Ln+accumulate on the scalar engine.
"""

import numpy as np
import ml_dtypes

B, G, T, K, P, S = 16, 4000, 96, 4, 500, 200
M = 8            # cores
GC = G // M      # 500 generators per core (BCE tiles)
BS = B // M      # 2 scenarios per core (B-sharded tensors)
GT = 4           # tile slots per core
BT = B * T       # 1536
SEG = T + 8      # padded segment (8 zero cols -> scan pad == clamp value)
BTP = B * SEG    # 1664
NCOL = 64
VIOLATIONS_PENALTY = 1000.0
POWER_BALANCE_PENALTY = 5000.0

# out_g column map (violation rows; accumulated with nsw = -sw sign)
C_SWON = 0       # sum nsw_on
C_GUP = 1        # sum nsw_on * (gup - cs)
C_GDN = 2        # sum nsw_off * (gdn - cs)
C_SWT0 = 16      # cols 16..22 : i=0..6 -> tau=7-i : sum_b nsw_on[b, T-tau]
C_XON0 = 23      # cols 23..29 : i -> tau=7-i : sum_b nsw_on[b,T-tau]*cs_raw[T-tau]
C_XOFF0 = 30     # cols 30..36
C_PFB0 = 37      # cols 37..45 : P_raw(r), r=0..8 (host: P(r)-P(0))
C_XTON0 = 46     # cols 46..52 : i -> tau=7-i : sum_b nsw_on[b,T-tau]*cs_raw[b,T]
C_XTOFF0 = 53    # cols 53..59

_NC_CACHE = {}


def _build_nc(plan_key):
    """plan_key = (tile_sizes, on_runs, off_runs) as nested tuples."""
    import concourse.bacc as bacc
    import concourse.tile as tile
    import concourse.mybir as mybir

    tile_sizes, on_runs, off_runs = plan_key
    rows_pc = sum(tile_sizes)

    f32 = mybir.dt.float32
    f16 = mybir.dt.float16
    f8 = mybir.dt.float8e4
    alu = mybir.AluOpType
    AX = mybir.AxisListType
    LN = mybir.ActivationFunctionType.Ln
    CP = mybir.ActivationFunctionType.Copy

    nc = bacc.Bacc("TRN2", target_bir_lowering=False, debug=False, num_devices=M)

    # const AP for the Ln(v + 0.5) bias used by the BCE select trick
    _half = nc.alloc_sbuf_tensor("const-half", [128, 1], f32)
    nc.gpsimd.memset(_half.ap(), 0.5)
    nc.const_aps.aps[(f32, 0.5)] = _half.ap()
    nc.all_engine_barrier()

    sE = nc.dram_tensor("sE", [rows_pc, BTP], f16, kind="ExternalInput").ap()
    pE = nc.dram_tensor("pE", [rows_pc, BTP], f16, kind="ExternalInput").ap()
    pp = nc.dram_tensor("pp", [GC, BT], f16, kind="ExternalInput").ap()
    tt = nc.dram_tensor("tt", [GC, BT], f16, kind="ExternalInput").ap()
    sp8 = nc.dram_tensor("sp8", [BT, GC * K], f8, kind="ExternalInput").ap()
    pg16 = nc.dram_tensor("pg16", [T, BS * P], f16, kind="ExternalInput").ap()
    st6 = nc.dram_tensor("st6", [T, 6 * S * BS + BS], f16, kind="ExternalInput").ap()
    ones16 = nc.dram_tensor("ones16", [128, 2], f16, kind="ExternalInput").ap()
    ones8 = nc.dram_tensor("ones8", [128, 2], f8, kind="ExternalInput").ap()

    out_g = nc.dram_tensor("out_g", [max(rows_pc, 1), NCOL], f32,
                           kind="ExternalOutput").ap()
    out_b = nc.dram_tensor("out_b", [GC, 2], f32, kind="ExternalOutput").ap()
    out_s = nc.dram_tensor("out_s", [T, 8], f32, kind="ExternalOutput").ap()
    out_pe = nc.dram_tensor("out_pe", [1, 3300], f32, kind="ExternalOutput").ap()

    # round-robin engines for gather DMA issue (keep sync queue light)
    def dma_engines():
        while True:
            yield nc.sync
            yield nc.scalar
    dmae = dma_engines()

    with tile.TileContext(nc) as tc:
        with (
            tc.tile_pool(name="inp", bufs=2) as inp,
            tc.tile_pool(name="work", bufs=2) as work,
            tc.tile_pool(name="colp", bufs=2) as colp,
            tc.tile_pool(name="cst", bufs=1) as cst,
            tc.tile_pool(name="ps", bufs=1, space="PSUM") as psp,
        ):
            o16 = cst.tile([128, 2], f16, tag="o16")
            nc.sync.dma_start(o16[:], ones16[:, :])
            o8 = cst.tile([128, 2], f8, tag="o8")
            nc.sync.dma_start(o8[:], ones8[:, :])

            # ---- seg_prod reduction on the tensor engine ----
            ps_seg = [psp.tile([1, 500], f32, tag=f"seg{q}", name=f"ps_seg{q}")
                      for q in range(4)]
            for ch in range(12):
                t8 = inp.tile([128, GC * K], f8, tag="sp")
                nc.sync.dma_start(t8[:], sp8[128 * ch:128 * (ch + 1), :])
                for q in range(4):
                    nc.tensor.matmul(
                        ps_seg[q][:], o8[:, 0:1], t8[:, 500 * q:500 * (q + 1)],
                        start=(ch == 0), stop=(ch == 11))

            # ---- profiled_generation row sums ----
            ps_pg = psp.tile([1, P], f32, tag="pg")
            pgt = inp.tile([T, BS * P], f16, tag="pg")
            nc.sync.dma_start(pgt[:], pg16[:, :])
            for b in range(2):
                nc.tensor.matmul(
                    ps_pg[:], o16[0:96, 0:1], pgt[:, P * b:P * (b + 1)],
                    start=(b == 0), stop=(b == 1))

            # ---- storage block ----
            SB = S * BS  # 400
            stt = inp.tile([T, 6 * SB + BS], f16, tag="st")
            nc.sync.dma_start(stt[:], st6[:, :])
            ps_cr = psp.tile([1, SB], f32, tag="cr")
            nc.tensor.matmul(ps_cr[:], o16[0:96, 0:1], stt[:, 4 * SB:5 * SB],
                             start=True, stop=True)
            ps_dr = psp.tile([1, SB], f32, tag="dr")
            nc.tensor.matmul(ps_dr[:], o16[0:96, 0:1], stt[:, 5 * SB:6 * SB],
                             start=True, stop=True)

            scols = colp.tile([T, 8], f32, tag="scols")
            nc.vector.memset(scols[:], 0.0)
            su = work.tile([T, SB], f16, tag="su")
            sv = work.tile([T, SB], f16, tag="sv")
            sq = work.tile([T, SB], f16, tag="sq")
            for i in range(2):
                pr = stt[:, (2 * i) * SB:(2 * i + 1) * SB]
                tg = stt[:, (2 * i + 1) * SB:(2 * i + 2) * SB]
                nc.scalar.activation(su[:], tg, CP, bias=-1.0, scale=2.0)
                nc.scalar.activation(sq[:], pr, CP, bias=-0.5, scale=1.0)
                nc.vector.tensor_tensor(out=sv[:], in0=sq[:], in1=su[:],
                                        op=alu.mult)
                nc.scalar.activation(sq[:], sv[:], LN, bias=0.5, scale=1.0,
                                     accum_out=scols[:, i:i + 1])
            nc.vector.tensor_reduce(
                scols[:, 2:3], stt[:, 6 * SB:6 * SB + BS], axis=AX.X, op=alu.add)
            nc.sync.dma_start(out_s[:, :], scols[:])

            # ---- assemble PE results ----
            peout = colp.tile([1, 3300], f32, tag="peout")
            for q in range(4):
                nc.scalar.activation(
                    peout[:, 500 * q:500 * (q + 1)], ps_seg[q][:], CP)
            nc.scalar.activation(peout[:, 2000:2500], ps_pg[:], CP)
            nc.scalar.activation(peout[:, 2500:2900], ps_cr[:], CP)
            nc.scalar.activation(peout[:, 2900:3300], ps_dr[:], CP)
            nc.sync.dma_start(out_pe[:, :], peout[:])

            # ---- BCE tiles (all 500 generators, unpermuted order) ----
            GB = GC // GT  # 125
            for it in range(GT):
                r0 = it * GB
                p_t = inp.tile([GB, BT], f16, tag="pp")
                nc.sync.dma_start(p_t[:], pp[r0:r0 + GB, :])
                t_t = inp.tile([GB, BT], f16, tag="tt")
                nc.sync.dma_start(t_t[:], tt[r0:r0 + GB, :])
                bcol = colp.tile([GB, 2], f32, tag="bcol")
                nc.vector.memset(bcol[:], 0.0)
                ub = work.tile([GB, BT], f16, tag="ub")
                p5 = work.tile([GB, BT], f16, tag="p5")
                nc.scalar.activation(ub[:], t_t[:], CP, bias=-1.0, scale=2.0)
                nc.scalar.activation(p5[:], p_t[:], CP, bias=-0.5, scale=1.0)
                vv = work.tile([GB, BT], f16, tag="vv")
                nc.vector.tensor_tensor(out=vv[:], in0=p5[:], in1=ub[:],
                                        op=alu.mult)
                nc.scalar.activation(ub[:], vv[:], LN, bias=0.5, scale=1.0,
                                     accum_out=bcol[:, 0:1])
                nc.sync.dma_start(out_b[r0:r0 + GB, :], bcol[:])

            # ---- violation tiles (class-aligned rows) ----
            r_base = 0
            for it in range(GT):
                GR = tile_sizes[it]
                r1g = r_base + GR
                se = inp.tile([128, BTP], f16, tag="se")
                nc.sync.dma_start(se[0:GR, :], sE[r_base:r1g, :])
                pe = inp.tile([128, BTP], f16, tag="pe")
                nc.sync.dma_start(pe[0:GR, :], pE[r_base:r1g, :])

                cols = colp.tile([128, NCOL], f32, tag="cols")
                nc.vector.memset(cols[:], 0.0)

                non = work.tile([128, BTP], f16, tag="non")
                nof = work.tile([128, BTP], f16, tag="nof")
                # nsw_on = (prev - 1) * s ; nsw_off = (s - 1) * prev
                # (pads: s=0, prev=0 -> both products 0)
                nc.vector.scalar_tensor_tensor(
                    out=non[0:GR], in0=pe[0:GR], scalar=1.0, in1=se[0:GR],
                    op0=alu.subtract, op1=alu.mult,
                    accum_out=cols[0:GR, C_SWON:C_SWON + 1])
                nc.vector.scalar_tensor_tensor(
                    out=nof[0:GR], in0=se[0:GR], scalar=1.0, in1=pe[0:GR],
                    op0=alu.subtract, op1=alu.mult)

                # prefix scan over the padded row; within pad cols the
                # running value sticks at cs_raw[b, T] (the clamp value)
                csx = work.tile([128, BTP + 8], f16, tag="csx")
                nc.vector.memset(csx[0:GR, 0:1], 0.0)
                nc.vector.memset(csx[0:GR, BTP + 1:BTP + 8], 0.0)
                nc.vector.tensor_tensor_scan(
                    out=csx[0:GR, 1:BTP + 1], data0=se[0:GR], data1=se[0:GR],
                    initial=0.0, op0=alu.add, op1=alu.bypass)

                # full-row shifted gathers (one contiguous line per row);
                # shifts never cross into the next segment thanks to the pad
                gup = work.tile([128, BTP], f16, tag="gup")
                gdn = work.tile([128, BTP], f16, tag="gdn")
                for dst, runs in ((gup, on_runs[it]), (gdn, off_runs[it])):
                    for (rr0, rr1, u) in runs:
                        next(dmae).dma_start(dst[rr0:rr1, :],
                                             csx[rr0:rr1, u:u + BTP])

                # dU = gup - cs ; dD = gdn - cs  (pads: finite garbage * 0)
                dU = work.tile([128, BTP], f16, tag="dU")
                nc.vector.tensor_sub(dU[0:GR], gup[0:GR], csx[0:GR, 0:BTP])
                nc.vector.scalar_tensor_tensor(
                    out=gup[0:GR], in0=non[0:GR], scalar=1.0, in1=dU[0:GR],
                    op0=alu.mult, op1=alu.mult,
                    accum_out=cols[0:GR, C_GUP:C_GUP + 1])
                nc.vector.tensor_sub(dU[0:GR], gdn[0:GR], csx[0:GR, 0:BTP])
                nc.vector.scalar_tensor_tensor(
                    out=gdn[0:GR], in0=nof[0:GR], scalar=1.0, in1=dU[0:GR],
                    op0=alu.mult, op1=alu.mult,
                    accum_out=cols[0:GR, C_GDN:C_GDN + 1])

                # corner columns (i=0..6 -> tau=7-i, t = T-tau = 89+i):
                #   XON/XOFF: sum_b nsw[b,t]*cs_raw[b,t]
                #   XT:       sum_b nsw[b,t]*cs_raw[b,T]
                nv = non[:].rearrange("g (b t) -> g b t", b=B)
                fv = nof[:].rearrange("g (b t) -> g b t", b=B)
                non7 = nv[0:GR, :, T - 7:T]
                nof7 = fv[0:GR, :, T - 7:T]
                csview = csx[:, 0:BTP].rearrange("g (b t) -> g b t", b=B)
                csx7 = csview[0:GR, :, T - 7:T]
                csxT = csview[0:GR, :, T:T + 1]
                prd = work.tile([128, B * 7], f16, tag="prd")
                pv = prd[:].rearrange("g (b i) -> g b i", b=B)
                pvr = prd[:].rearrange("g (b i) -> g i b", b=B)

                for (nsw7, c_x, c_xt) in ((non7, C_XON0, C_XTON0),
                                          (nof7, C_XOFF0, C_XTOFF0)):
                    nc.vector.tensor_mul(pv[0:GR], nsw7, csx7)
                    nc.vector.tensor_reduce(
                        cols[0:GR, c_x:c_x + 7], pvr[0:GR],
                        axis=AX.X, op=alu.add)
                    nc.vector.tensor_mul(pv[0:GR], nsw7,
                                         csxT.to_broadcast((GR, B, 7)))
                    nc.vector.tensor_reduce(
                        cols[0:GR, c_xt:c_xt + 7], pvr[0:GR],
                        axis=AX.X, op=alu.add)

                # SWT: sum_b nsw_on[b, 89+i]
                non7r = non[:].rearrange("g (b t) -> g t b", b=B)[0:GR, T - 7:T, :]
                nc.vector.tensor_reduce(
                    cols[0:GR, C_SWT0:C_SWT0 + 7], non7r, axis=AX.X, op=alu.add)

                # prefix columns from csx: P_raw(r) = sum_b csx[SEG*b + r]
                nc.vector.tensor_reduce(
                    cols[0:GR, C_PFB0:C_PFB0 + 9],
                    csx[0:GR, 0:BTP].rearrange(
                        "g (b r) -> g r b", b=B)[:, 0:9, :],
                    axis=AX.X, op=alu.add)

                nc.sync.dma_start(out_g[r_base:r1g, :], cols[0:GR])
                r_base = r1g

    nc.compile()
    return nc


def _get_nc(plan_key):
    if plan_key not in _NC_CACHE:
        _NC_CACHE[plan_key] = _build_nc(plan_key)
    return _NC_CACHE[plan_key]


def _plan(U, D):
    """Class-aligned row plan.

    Returns (plan_key, rows[core] -> list of g ids (len rows_pc),
    host_g -> leftover g ids).
    """
    idx = {}
    for g in range(G):
        idx.setdefault((int(U[g]), int(D[g])), []).append(g)
    # snake order: u desc; d direction alternates per u-block so adjacent
    # blocks share d at the boundary (merges OFF gather runs)
    u_vals = sorted({u for (u, _) in idx.keys()}, reverse=True)
    classes = []
    for k, u in enumerate(u_vals):
        ds = sorted({d for (uu, d) in idx.keys() if uu == u}, reverse=(k % 2 == 0))
        classes.extend((u, d) for d in ds)
    rows = [[] for _ in range(M)]
    host_g = []
    spans = []  # (row0, row1, u, d) in per-core row space
    r = 0
    for ud in classes:
        ids = idx[ud]
        n = len(ids) // M
        if n:
            for c in range(M):
                rows[c].extend(ids[c * n:(c + 1) * n])
            spans.append((r, r + n, ud[0], ud[1]))
            r += n
        host_g.extend(ids[M * n:])
    rows_pc = r
    # tile sizes: as equal as possible, <= 128
    nt = GT
    base = rows_pc // nt
    sizes = [base + (1 if i < rows_pc % nt else 0) for i in range(nt)]
    assert max(sizes) <= 128, sizes
    # split spans at tile boundaries; build per-tile run lists
    bounds = np.cumsum([0] + sizes)
    on_runs = [[] for _ in range(nt)]
    off_runs = [[] for _ in range(nt)]
    for (s0, s1, u, d) in spans:
        for ti in range(nt):
            lo = max(s0, int(bounds[ti]))
            hi = min(s1, int(bounds[ti + 1]))
            if lo < hi:
                t0 = int(bounds[ti])
                off_runs[ti].append((lo - t0, hi - t0, d))
                # merge ON runs with same shift
                if on_runs[ti] and on_runs[ti][-1][1] == lo - t0 \
                        and on_runs[ti][-1][2] == u:
                    on_runs[ti][-1] = (on_runs[ti][-1][0], hi - t0, u)
                else:
                    on_runs[ti].append((lo - t0, hi - t0, u))
    # merge OFF runs with same consecutive shift too
    for ti in range(nt):
        merged = []
        for run in off_runs[ti]:
            if merged and merged[-1][1] == run[0] and merged[-1][2] == run[2]:
                merged[-1] = (merged[-1][0], run[1], run[2])
            else:
                merged.append(run)
        off_runs[ti] = merged
        on_runs[ti] = [tuple(x) for x in on_runs[ti]]
    plan_key = (tuple(sizes),
                tuple(tuple(rl) for rl in on_runs),
                tuple(tuple(rl) for rl in off_runs))
    return plan_key, rows, host_g


def _f16(a):
    return np.ascontiguousarray(a, dtype=np.float16)


def _prep_in_maps(inputs, rows):
    ic = np.asarray(inputs["initial_commitment"], dtype=np.float32)      # (B,G)
    s_full = np.asarray(inputs["thermal_on_rounded"], dtype=np.float32)  # (B,G,T)
    p_full = np.asarray(inputs["thermal_on"], dtype=np.float32)
    t_full = np.asarray(inputs["tgt_thermal_commitment"], dtype=np.float32)
    sp_full = np.asarray(inputs["seg_prod"], dtype=np.float32)           # (B,G,T,K)
    pg_full = np.asarray(inputs["profiled_generation"], dtype=np.float32)
    chp = np.asarray(inputs["is_charging"], dtype=np.float32)            # (B,S,T)
    cht = np.asarray(inputs["tgt_is_charging"], dtype=np.float32)
    dsp = np.asarray(inputs["is_discharging"], dtype=np.float32)
    dst = np.asarray(inputs["tgt_is_discharging"], dtype=np.float32)
    cr = np.asarray(inputs["charge_rate"], dtype=np.float32)
    dr = np.asarray(inputs["discharge_rate"], dtype=np.float32)
    curt = np.asarray(inputs["curtailment"], dtype=np.float32)           # (B,T)

    ones16 = np.ones((128, 2), dtype=np.float16)
    ones8 = np.ones((128, 2), dtype=ml_dtypes.float8_e4m3)

    in_maps = []
    for c in range(M):
        gids = np.asarray(rows[c], dtype=np.int64)
        s = s_full[:, gids, :].transpose(1, 0, 2)            # (rows,B,T)
        n = len(gids)
        sEc = np.zeros((n, B, SEG), dtype=np.float16)
        sEc[:, :, :T] = s
        pEc = np.zeros((n, B, SEG), dtype=np.float16)
        pEc[:, :, 0] = ic[:, gids].T
        pEc[:, :, 1:T] = s[:, :, :T - 1]

        gsl = slice(GC * c, GC * (c + 1))
        bsl = slice(BS * c, BS * (c + 1))
        sp8c = np.ascontiguousarray(
            sp_full[:, gsl].transpose(0, 2, 1, 3).reshape(BT, GC * K)
        ).astype(ml_dtypes.float8_e4m3)
        # (BS,P,T) -> (T, BS*P), col = b*P + p
        pgc = _f16(pg_full[bsl].transpose(2, 0, 1).reshape(T, BS * P))

        def sb(x):  # (BS,S,T) -> (T, S*BS) with col = s*BS + b
            return x[bsl].transpose(2, 1, 0).reshape(T, S * BS)

        st6c = np.concatenate(
            [sb(chp), sb(cht), sb(dsp), sb(dst), sb(cr), sb(dr),
             curt[bsl].T], axis=1)

        in_maps.append({
            "sE": sEc.reshape(n, BTP),
            "pE": pEc.reshape(n, BTP),
            "pp": _f16(p_full[:, gsl].transpose(1, 0, 2)).reshape(GC, BT),
            "tt": _f16(t_full[:, gsl].transpose(1, 0, 2)).reshape(GC, BT),
            "sp8": sp8c,
            "pg16": pgc,
            "st6": _f16(st6c),
            "ones16": ones16,
            "ones8": ones8,
        })
    return in_maps


def kernel(**inputs):
    from concourse.bass_utils import run_bass_kernel_spmd

    U_all = np.maximum(np.asarray(inputs["min_uptimes"]).astype(np.int64), 0)
    D_all = np.maximum(np.asarray(inputs["min_downtimes"]).astype(np.int64), 0)
    plan_key, rows, host_g = _plan(U_all, D_all)
    nc = _get_nc(plan_key)
    in_maps = _prep_in_maps(inputs, rows)
    res = run_bass_kernel_spmd(nc, in_maps, core_ids=list(range(M)))
    return _combine(res.results, inputs, rows, host_g)


def _host_viol(inputs, gids):
    """Reference-exact violations + switch-on counts for leftover g ids."""
    if len(gids) == 0:
        return 0.0, np.zeros(0)
    gids = np.asarray(gids, dtype=np.int64)
    s = np.asarray(inputs["thermal_on_rounded"], dtype=np.float64)[:, gids, :]
    ic = np.asarray(inputs["initial_commitment"], dtype=np.float64)[:, gids]
    U = np.maximum(np.asarray(inputs["min_uptimes"]).astype(np.int64)[gids], 0)
    D = np.maximum(np.asarray(inputs["min_downtimes"]).astype(np.int64)[gids], 0)
    stat = np.asarray(inputs["initial_status"]).astype(np.int64)[gids]

    prev = np.concatenate([ic[:, :, None], s[:, :, :-1]], axis=2)
    sw_on = (1.0 - prev) * s
    sw_off = prev * (1.0 - s)

    def mtv(sw, series, early_series, W, remaining):
        Bn, Gn, Tn = series.shape
        t = np.arange(Tn)
        early = (t[None, None, :] < remaining[None, :, None]) * early_series
        cs = np.concatenate([np.zeros((Bn, Gn, 1)), np.cumsum(series, axis=-1)],
                            axis=-1)
        end = t[None, :] + W[:, None]
        idx = np.clip(end, 0, Tn)
        wsum = np.take_along_axis(
            cs, np.broadcast_to(idx[None], (Bn, Gn, Tn)), axis=-1) - cs[:, :, :Tn]
        valid = ((end <= Tn) & (W[:, None] > 0))[None]
        viol = np.maximum(W[:, None][None] - wsum, 0.0) * valid * sw
        return early + viol

    rem_up = np.maximum(U - np.maximum(stat, 0), 0)
    rem_dn = np.maximum(D - np.maximum(-stat, 0), 0)
    up_v = mtv(sw_on, s, 1.0 - s, U, rem_up)
    dn_v = mtv(sw_off, 1.0 - s, s, D, rem_dn)
    swon_per_g = sw_on.sum(axis=(0, 2))
    return float(up_v.sum() + dn_v.sum()), swon_per_g


def _combine(results, inputs, rows, host_g):
    U_all = np.asarray(inputs["min_uptimes"]).astype(np.int64)
    D_all = np.asarray(inputs["min_downtimes"]).astype(np.int64)
    stat_all = np.asarray(inputs["initial_status"]).astype(np.int64)
    suc_all = np.asarray(inputs["start_up_costs"], dtype=np.float64)
    segc_all = np.asarray(inputs["segment_cost"], dtype=np.float64)[:, 0, :]
    puc = np.asarray(inputs["profiled_units_cost"], dtype=np.float64)
    ccost = np.asarray(inputs["charge_costs"], dtype=np.float64)
    dcost = np.asarray(inputs["discharge_costs"], dtype=np.float64)

    tt_i = 7 - np.arange(7)[None, :]       # column i -> tau = 7-i

    viol = 0.0
    ed = 0.0
    bce_th = 0.0
    bce_ch = 0.0
    bce_ds = 0.0
    curt_sum = 0.0

    for c in range(M):
        o = np.asarray(results[c]["out_g"], dtype=np.float64)
        ob = np.asarray(results[c]["out_b"], dtype=np.float64)
        ope = np.asarray(results[c]["out_pe"], dtype=np.float64)[0]
        osr = np.asarray(results[c]["out_s"], dtype=np.float64)

        gids = np.asarray(rows[c], dtype=np.int64)
        n = len(gids)
        o = o[:n]
        U = U_all[gids]
        D = D_all[gids]
        stat = stat_all[gids]

        SWON = -o[:, C_SWON]
        A_on = o[:, C_GUP] - U * o[:, C_SWON]   # = U*SWON - sum sw*dU
        A_off = -o[:, C_GDN]
        SWT = -o[:, C_SWT0:C_SWT0 + 7]
        XON = -o[:, C_XON0:C_XON0 + 7]
        XOFF = -o[:, C_XOFF0:C_XOFF0 + 7]
        XTON = -o[:, C_XTON0:C_XTON0 + 7]
        XTOFF = -o[:, C_XTOFF0:C_XTOFF0 + 7]
        Praw = o[:, C_PFB0:C_PFB0 + 9]
        Pf = Praw - Praw[:, 0:1]

        mU = (tt_i < U[:, None])
        mD = (tt_i < D[:, None])
        up = A_on.sum()
        up -= (U[:, None] * SWT * mU).sum()
        up += (XTON * mU).sum()
        up -= (XON * mU).sum()
        dn = A_off.sum()
        dn -= (XTOFF * mD).sum()
        dn += (XOFF * mD).sum()
        rem_up = np.maximum(U - np.maximum(stat, 0), 0)
        rem_dn = np.maximum(D - np.maximum(-stat, 0), 0)
        g_idx = np.arange(n)
        early = (B * rem_up - Pf[g_idx, rem_up]).sum() + Pf[g_idx, rem_dn].sum()
        viol += up + dn + early

        ed += (suc_all[gids] * SWON).sum()
        bce_th += ob[:, 0].sum()

        gsl = slice(GC * c, GC * (c + 1))
        ed += (segc_all[gsl].reshape(-1) * ope[0:2000]).sum()
        ed += (puc * ope[2000:2500]).sum()
        ed += (ccost * ope[2500:2900].reshape(S, BS).sum(axis=1)).sum()
        ed += (dcost * ope[2900:3300].reshape(S, BS).sum(axis=1)).sum()

        bce_ch += osr[:, 0].sum()
        bce_ds += osr[:, 1].sum()
        curt_sum += osr[:, 2].sum()

    # leftover generators fully on host (violations + startup term)
    hviol, hswon = _host_viol(inputs, host_g)
    viol += hviol
    if len(host_g):
        ed += (suc_all[np.asarray(host_g, dtype=np.int64)] * hswon).sum()

    n_th = float(B * G * T)
    n_s = float(B * S * T)
    sup = -(bce_th / n_th) - (bce_ch / n_s) - (bce_ds / n_s)
    total = (ed + POWER_BALANCE_PENALTY * curt_sum + sup
             + VIOLATIONS_PENALTY * viol)
    return np.float32(total)


# revision 17
# speedup vs baseline: 1.1423x; 1.0352x over previous
"""Trainium2 Bass kernel for the unit-commitment custom loss (v6).

Strategy (8 NeuronCores, SPMD single program):
  - Min-up/down-time violations via ONE per-row prefix scan plus staged
    per-row ROTATED copies of the commitment series. Each generator row is
    staged in a padded layout ([B, 104] per row: 96 time steps + 8 zero
    pad), and additionally staged shifted right by its own U (resp. D).
    The scan of the unshifted copy then lines up positionally:
        sum_t sw_on[t] * (cs[t+U] - cs[t])
          = sum_p (nsw_onU - nsw_on0)[p] * cs_scan[p]
    so each side needs only one subtract + one fused product-accumulate -
    no shift-dependent passes, no gathers, no data-dependent program
    structure. The scan's zero-pad region holds cs[T], giving the
    window-clamp for free; host corner corrections use the XON/XT columns.
  - All elementwise work is fp16 (DVE 2x tensor_tensor mode; counts
    <= 2048 exact in fp16); affine preps run on the scalar engine.
  - seg_prod is staged fp8e4m3 as [B*T, G*K] and reduced on the idle
    tensor engine with ones-vector matmuls, as are profiled_generation
    and charge/discharge rate row sums.
  - BCE terms use the binary-target select trick: q = 0.5+(2t-1)(p-0.5),
    one Ln+accumulate per tensor pair on the scalar engine.
"""

import numpy as np
import ml_dtypes

B, G, T, K, P, S = 16, 4000, 96, 4, 500, 200
M = 8            # cores
GC = G // M      # 500 generators per core
BS = B // M      # 2 scenarios per core (B-sharded tensors)
GT = 4           # tile slots per core
GR = GC // GT    # 125 rows per tile
BT = B * T       # 1536
SEG = T + 8      # padded segment; scan pad == clamp value cs[T]
BTP = B * SEG    # 1664
NCOL = 64
VIOLATIONS_PENALTY = 1000.0
POWER_BALANCE_PENALTY = 5000.0

# out_g column map (accumulated with nsw = -sw sign)
C_SWON = 0       # sum nsw_on
C_GUP = 1        # sum (nsw_onU - nsw_on0) * cs   (= sum sw_on*(cs[t]-cs[t+U]))
C_GDN = 2        # sum (nsw_offD - nsw_off0) * cs
C_SWT0 = 16      # cols 16..22 : i=0..6 -> tau=7-i : sum_b nsw_on[b, T-tau]
C_XON0 = 23      # cols 23..29 : sum_b nsw_on[b,T-tau]*cs_raw[b,T-tau]
C_XOFF0 = 30     # cols 30..36
C_PFB0 = 37      # cols 37..45 : P_raw(r), r=0..8 (host: P(r)-P(0))
C_XTON0 = 46     # cols 46..52 : sum_b nsw_on[b,T-tau]*cs_raw[b,T]
C_XTOFF0 = 53    # cols 53..59

_NC = None


def _build_nc():
    import concourse.bacc as bacc
    import concourse.tile as tile
    import concourse.mybir as mybir

    f32 = mybir.dt.float32
    f16 = mybir.dt.float16
    f8 = mybir.dt.float8e4
    alu = mybir.AluOpType
    AX = mybir.AxisListType
    LN = mybir.ActivationFunctionType.Ln
    CP = mybir.ActivationFunctionType.Copy

    nc = bacc.Bacc("TRN2", target_bir_lowering=False, debug=False, num_devices=M)

    # const AP for the Ln(v + 0.5) bias used by the BCE select trick
    _half = nc.alloc_sbuf_tensor("const-half", [128, 1], f32)
    nc.gpsimd.memset(_half.ap(), 0.5)
    nc.const_aps.aps[(f32, 0.5)] = _half.ap()
    nc.all_engine_barrier()

    sE0 = nc.dram_tensor("sE0", [GC, BTP], f16, kind="ExternalInput").ap()
    pE0 = nc.dram_tensor("pE0", [GC, BTP], f16, kind="ExternalInput").ap()
    sEU = nc.dram_tensor("sEU", [GC, BTP], f16, kind="ExternalInput").ap()
    pEU = nc.dram_tensor("pEU", [GC, BTP], f16, kind="ExternalInput").ap()
    sED = nc.dram_tensor("sED", [GC, BTP], f16, kind="ExternalInput").ap()
    pED = nc.dram_tensor("pED", [GC, BTP], f16, kind="ExternalInput").ap()
    pp = nc.dram_tensor("pp", [GC, BT], f16, kind="ExternalInput").ap()
    tt = nc.dram_tensor("tt", [GC, BT], f16, kind="ExternalInput").ap()
    sp8 = nc.dram_tensor("sp8", [BT, GC * K], f8, kind="ExternalInput").ap()
    pg16 = nc.dram_tensor("pg16", [T, BS * P], f16, kind="ExternalInput").ap()
    st6 = nc.dram_tensor("st6", [T, 6 * S * BS + BS], f16, kind="ExternalInput").ap()
    ones16 = nc.dram_tensor("ones16", [128, 2], f16, kind="ExternalInput").ap()
    ones8 = nc.dram_tensor("ones8", [128, 2], f8, kind="ExternalInput").ap()

    out_g = nc.dram_tensor("out_g", [GC, NCOL], f32, kind="ExternalOutput").ap()
    out_b = nc.dram_tensor("out_b", [GC, 2], f32, kind="ExternalOutput").ap()
    out_s = nc.dram_tensor("out_s", [T, 8], f32, kind="ExternalOutput").ap()
    out_pe = nc.dram_tensor("out_pe", [1, 3300], f32, kind="ExternalOutput").ap()

    with tile.TileContext(nc) as tc:
        with (
            tc.tile_pool(name="inp", bufs=2) as inp,
            tc.tile_pool(name="work", bufs=2) as work,
            tc.tile_pool(name="colp", bufs=2) as colp,
            tc.tile_pool(name="cst", bufs=1) as cst,
            tc.tile_pool(name="ps", bufs=1, space="PSUM") as psp,
        ):
            o16 = cst.tile([128, 2], f16, tag="o16")
            nc.sync.dma_start(o16[:], ones16[:, :])
            o8 = cst.tile([128, 2], f8, tag="o8")
            nc.sync.dma_start(o8[:], ones8[:, :])

            # ---- seg_prod reduction on the tensor engine ----
            ps_seg = [psp.tile([1, 500], f32, tag=f"seg{q}", name=f"ps_seg{q}")
                      for q in range(4)]
            for ch in range(12):
                t8 = inp.tile([128, GC * K], f8, tag="sp")
                nc.sync.dma_start(t8[:], sp8[128 * ch:128 * (ch + 1), :])
                for q in range(4):
                    nc.tensor.matmul(
                        ps_seg[q][:], o8[:, 0:1], t8[:, 500 * q:500 * (q + 1)],
                        start=(ch == 0), stop=(ch == 11))

            # ---- profiled_generation row sums ----
            ps_pg = psp.tile([1, P], f32, tag="pg")
            pgt = inp.tile([T, BS * P], f16, tag="pg")
            nc.sync.dma_start(pgt[:], pg16[:, :])
            for b in range(2):
                nc.tensor.matmul(
                    ps_pg[:], o16[0:96, 0:1], pgt[:, P * b:P * (b + 1)],
                    start=(b == 0), stop=(b == 1))

            # ---- storage block ----
            SB = S * BS  # 400
            stt = inp.tile([T, 6 * SB + BS], f16, tag="st")
            nc.sync.dma_start(stt[:], st6[:, :])
            ps_cr = psp.tile([1, SB], f32, tag="cr")
            nc.tensor.matmul(ps_cr[:], o16[0:96, 0:1], stt[:, 4 * SB:5 * SB],
                             start=True, stop=True)
            ps_dr = psp.tile([1, SB], f32, tag="dr")
            nc.tensor.matmul(ps_dr[:], o16[0:96, 0:1], stt[:, 5 * SB:6 * SB],
                             start=True, stop=True)

            scols = colp.tile([T, 8], f32, tag="scols")
            nc.vector.memset(scols[:], 0.0)
            su = work.tile([T, SB], f16, tag="su")
            sv = work.tile([T, SB], f16, tag="sv")
            sq = work.tile([T, SB], f16, tag="sq")
            for i in range(2):
                pr = stt[:, (2 * i) * SB:(2 * i + 1) * SB]
                tg = stt[:, (2 * i + 1) * SB:(2 * i + 2) * SB]
                nc.scalar.activation(su[:], tg, CP, bias=-1.0, scale=2.0)
                nc.scalar.activation(sq[:], pr, CP, bias=-0.5, scale=1.0)
                nc.vector.tensor_tensor(out=sv[:], in0=sq[:], in1=su[:],
                                        op=alu.mult)
                nc.scalar.activation(sq[:], sv[:], LN, bias=0.5, scale=1.0,
                                     accum_out=scols[:, i:i + 1])
            nc.vector.tensor_reduce(
                scols[:, 2:3], stt[:, 6 * SB:6 * SB + BS], axis=AX.X, op=alu.add)
            nc.sync.dma_start(out_s[:, :], scols[:])

            # ---- assemble PE results ----
            peout = colp.tile([1, 3300], f32, tag="peout")
            for q in range(4):
                nc.scalar.activation(
                    peout[:, 500 * q:500 * (q + 1)], ps_seg[q][:], CP)
            nc.scalar.activation(peout[:, 2000:2500], ps_pg[:], CP)
            nc.scalar.activation(peout[:, 2500:2900], ps_cr[:], CP)
            nc.scalar.activation(peout[:, 2900:3300], ps_dr[:], CP)
            nc.sync.dma_start(out_pe[:, :], peout[:])

            # ---- BCE tiles ----
            for it in range(GT):
                r0 = it * GR
                p_t = inp.tile([GR, BT], f16, tag="pp")
                nc.sync.dma_start(p_t[:], pp[r0:r0 + GR, :])
                t_t = inp.tile([GR, BT], f16, tag="tt")
                nc.sync.dma_start(t_t[:], tt[r0:r0 + GR, :])
                bcol = colp.tile([GR, 2], f32, tag="bcol")
                nc.vector.memset(bcol[:], 0.0)
                ub = work.tile([GR, BT], f16, tag="ub")
                p5 = work.tile([GR, BT], f16, tag="p5")
                nc.scalar.activation(ub[:], t_t[:], CP, bias=-1.0, scale=2.0)
                nc.scalar.activation(p5[:], p_t[:], CP, bias=-0.5, scale=1.0)
                vv = work.tile([GR, BT], f16, tag="vv")
                nc.vector.tensor_tensor(out=vv[:], in0=p5[:], in1=ub[:],
                                        op=alu.mult)
                nc.scalar.activation(ub[:], vv[:], LN, bias=0.5, scale=1.0,
                                     accum_out=bcol[:, 0:1])
                nc.sync.dma_start(out_b[r0:r0 + GR, :], bcol[:])

            # ---- violation tiles ----
            for it in range(GT):
                r0 = it * GR
                r1 = r0 + GR
                se0 = inp.tile([GR, BTP], f16, tag="se0")
                nc.sync.dma_start(se0[:], sE0[r0:r1, :])
                pe0 = inp.tile([GR, BTP], f16, tag="pe0")
                nc.sync.dma_start(pe0[:], pE0[r0:r1, :])
                seU = inp.tile([GR, BTP], f16, tag="seU")
                nc.sync.dma_start(seU[:], sEU[r0:r1, :])
                peU = inp.tile([GR, BTP], f16, tag="peU")
                nc.sync.dma_start(peU[:], pEU[r0:r1, :])
                seD = inp.tile([GR, BTP], f16, tag="seD")
                nc.sync.dma_start(seD[:], sED[r0:r1, :])
                peD = inp.tile([GR, BTP], f16, tag="peD")
                nc.sync.dma_start(peD[:], pED[r0:r1, :])

                cols = colp.tile([GR, NCOL], f32, tag="cols")
                nc.vector.memset(cols[:], 0.0)

                # nsw_on0 = (prev0 - 1) * s0 (STT, with SWON accum)
                non0 = work.tile([GR, BTP], f16, tag="non0")
                nc.vector.scalar_tensor_tensor(
                    out=non0[:], in0=pe0[:], scalar=1.0, in1=se0[:],
                    op0=alu.subtract, op1=alu.mult,
                    accum_out=cols[:, C_SWON:C_SWON + 1])
                # remaining builds: scalar-engine (x-1) prep + DVE 2x multiply
                prep = work.tile([GR, BTP], f16, tag="prep")
                nof0 = work.tile([GR, BTP], f16, tag="nof0")
                nc.scalar.activation(prep[:], se0[:], CP, bias=-1.0, scale=1.0)
                nc.vector.tensor_mul(nof0[:], prep[:], pe0[:])
                dlt_on = work.tile([GR, BTP], f16, tag="dlt_on")
                nc.scalar.activation(prep[:], peU[:], CP, bias=-1.0, scale=1.0)
                nc.vector.tensor_mul(dlt_on[:], prep[:], seU[:])
                dlt_off = work.tile([GR, BTP], f16, tag="dlt_off")
                nc.scalar.activation(prep[:], seD[:], CP, bias=-1.0, scale=1.0)
                nc.vector.tensor_mul(dlt_off[:], prep[:], peD[:])
                # deltas: nswU - nsw0 (exact small values in fp16)
                nc.vector.tensor_sub(dlt_on[:], dlt_on[:], non0[:])
                nc.vector.tensor_sub(dlt_off[:], dlt_off[:], nof0[:])

                # prefix scan of the unshifted copy
                csx = work.tile([GR, BTP + 2], f16, tag="csx")
                nc.vector.memset(csx[:, 0:1], 0.0)
                nc.vector.memset(csx[:, BTP + 1:BTP + 2], 0.0)
                nc.vector.tensor_tensor_scan(
                    out=csx[:, 1:BTP + 1], data0=se0[:], data1=se0[:],
                    initial=0.0, op0=alu.add, op1=alu.bypass)

                # main accumulations (product scratch goes to prep)
                nc.vector.scalar_tensor_tensor(
                    out=prep[:], in0=dlt_on[:], scalar=1.0,
                    in1=csx[:, 0:BTP], op0=alu.mult, op1=alu.mult,
                    accum_out=cols[:, C_GUP:C_GUP + 1])
                nc.vector.scalar_tensor_tensor(
                    out=prep[:], in0=dlt_off[:], scalar=1.0,
                    in1=csx[:, 0:BTP], op0=alu.mult, op1=alu.mult,
                    accum_out=cols[:, C_GDN:C_GDN + 1])

                # corner columns (i=0..6 -> tau=7-i, t = T-tau = 89+i)
                nv = non0[:].rearrange("g (b t) -> g b t", b=B)
                fv = nof0[:].rearrange("g (b t) -> g b t", b=B)
                non7 = nv[:, :, T - 7:T]
                nof7 = fv[:, :, T - 7:T]
                csview = csx[:, 0:BTP].rearrange("g (b t) -> g b t", b=B)
                csx7 = csview[:, :, T - 7:T]
                csxT = csview[:, :, T:T + 1]
                prd = work.tile([GR, B * 7], f16, tag="prd")
                pv = prd[:].rearrange("g (b i) -> g b i", b=B)
                pvr = prd[:].rearrange("g (b i) -> g i b", b=B)

                for (nsw7, c_x, c_xt) in ((non7, C_XON0, C_XTON0),
                                          (nof7, C_XOFF0, C_XTOFF0)):
                    nc.vector.tensor_mul(pv[:], nsw7, csx7)
                    nc.vector.tensor_reduce(
                        cols[:, c_x:c_x + 7], pvr[:], axis=AX.X, op=alu.add)
                    nc.vector.tensor_mul(pv[:], nsw7,
                                         csxT.to_broadcast((GR, B, 7)))
                    nc.vector.tensor_reduce(
                        cols[:, c_xt:c_xt + 7], pvr[:], axis=AX.X, op=alu.add)

                # SWT: sum_b nsw_on[b, 89+i]
                non7r = non0[:].rearrange("g (b t) -> g t b", b=B)[:, T - 7:T, :]
                nc.vector.tensor_reduce(
                    cols[:, C_SWT0:C_SWT0 + 7], non7r, axis=AX.X, op=alu.add)

                # prefix columns: P_raw(r) = sum_b csx[SEG*b + r]
                nc.vector.tensor_reduce(
                    cols[:, C_PFB0:C_PFB0 + 9],
                    csx[:, 0:BTP].rearrange(
                        "g (b r) -> g r b", b=B)[:, 0:9, :],
                    axis=AX.X, op=alu.add)

                nc.sync.dma_start(out_g[r0:r1, :], cols[:])

    nc.compile()
    return nc


def _get_nc():
    global _NC
    if _NC is None:
        _NC = _build_nc()
    return _NC


def _f16(a):
    return np.ascontiguousarray(a, dtype=np.float16)


def _prep_in_maps(inputs):
    ic = np.asarray(inputs["initial_commitment"], dtype=np.float32)      # (B,G)
    s_full = np.asarray(inputs["thermal_on_rounded"], dtype=np.float32)  # (B,G,T)
    p_full = np.asarray(inputs["thermal_on"], dtype=np.float32)
    t_full = np.asarray(inputs["tgt_thermal_commitment"], dtype=np.float32)
    sp_full = np.asarray(inputs["seg_prod"], dtype=np.float32)           # (B,G,T,K)
    pg_full = np.asarray(inputs["profiled_generation"], dtype=np.float32)
    chp = np.asarray(inputs["is_charging"], dtype=np.float32)            # (B,S,T)
    cht = np.asarray(inputs["tgt_is_charging"], dtype=np.float32)
    dsp = np.asarray(inputs["is_discharging"], dtype=np.float32)
    dst = np.asarray(inputs["tgt_is_discharging"], dtype=np.float32)
    cr = np.asarray(inputs["charge_rate"], dtype=np.float32)
    dr = np.asarray(inputs["discharge_rate"], dtype=np.float32)
    curt = np.asarray(inputs["curtailment"], dtype=np.float32)           # (B,T)
    U_all = np.maximum(np.asarray(inputs["min_uptimes"]).astype(np.int64), 0)
    D_all = np.maximum(np.asarray(inputs["min_downtimes"]).astype(np.int64), 0)

    ones16 = np.ones((128, 2), dtype=np.float16)
    ones8 = np.ones((128, 2), dtype=ml_dtypes.float8_e4m3)

    in_maps = []
    for c in range(M):
        gsl = slice(GC * c, GC * (c + 1))
        s = s_full[:, gsl, :].transpose(1, 0, 2).astype(np.float16)  # (GC,B,T)
        prev = np.empty((GC, B, T), dtype=np.float16)
        prev[:, :, 0] = ic[:, gsl].T
        prev[:, :, 1:] = s[:, :, :T - 1]
        U = np.minimum(U_all[gsl], 8)
        D = np.minimum(D_all[gsl], 8)

        def pack(x, shift):
            out = np.zeros((GC, B, SEG), dtype=np.float16)
            if shift is None:
                out[:, :, :T] = x
            else:
                for u in range(1, 9):
                    m = shift == u
                    if m.any():
                        out[m, :, u:u + T] = x[m]
            return out.reshape(GC, BTP)

        sp8c = np.ascontiguousarray(
            sp_full[:, gsl].transpose(0, 2, 1, 3).reshape(BT, GC * K)
        ).astype(ml_dtypes.float8_e4m3)
        pgc = _f16(pg_full[bsl_ := slice(BS * c, BS * (c + 1))]
                   .transpose(2, 0, 1).reshape(T, BS * P))

        def sb(x):  # (BS,S,T) -> (T, S*BS) with col = s*BS + b
            return x[bsl_].transpose(2, 1, 0).reshape(T, S * BS)

        st6c = np.concatenate(
            [sb(chp), sb(cht), sb(dsp), sb(dst), sb(cr), sb(dr),
             curt[bsl_].T], axis=1)

        in_maps.append({
            "sE0": pack(s, None),
            "pE0": pack(prev, None),
            "sEU": pack(s, U),
            "pEU": pack(prev, U),
            "sED": pack(s, D),
            "pED": pack(prev, D),
            "pp": _f16(p_full[:, gsl].transpose(1, 0, 2)).reshape(GC, BT),
            "tt": _f16(t_full[:, gsl].transpose(1, 0, 2)).reshape(GC, BT),
            "sp8": sp8c,
            "pg16": pgc,
            "st6": _f16(st6c),
            "ones16": ones16,
            "ones8": ones8,
        })
    return in_maps


def kernel(**inputs):
    from concourse.bass_utils import run_bass_kernel_spmd

    nc = _get_nc()
    in_maps = _prep_in_maps(inputs)
    res = run_bass_kernel_spmd(nc, in_maps, core_ids=list(range(M)))
    return _combine(res.results, inputs)


def _combine(results, inputs):
    U_all = np.minimum(
        np.maximum(np.asarray(inputs["min_uptimes"]).astype(np.int64), 0), 8)
    D_all = np.minimum(
        np.maximum(np.asarray(inputs["min_downtimes"]).astype(np.int64), 0), 8)
    stat_all = np.asarray(inputs["initial_status"]).astype(np.int64)
    suc_all = np.asarray(inputs["start_up_costs"], dtype=np.float64)
    segc_all = np.asarray(inputs["segment_cost"], dtype=np.float64)[:, 0, :]
    puc = np.asarray(inputs["profiled_units_cost"], dtype=np.float64)
    ccost = np.asarray(inputs["charge_costs"], dtype=np.float64)
    dcost = np.asarray(inputs["discharge_costs"], dtype=np.float64)

    tt_i = 7 - np.arange(7)[None, :]       # column i -> tau = 7-i

    viol = 0.0
    ed = 0.0
    bce_th = 0.0
    bce_ch = 0.0
    bce_ds = 0.0
    curt_sum = 0.0

    for c in range(M):
        o = np.asarray(results[c]["out_g"], dtype=np.float64)
        ob = np.asarray(results[c]["out_b"], dtype=np.float64)
        ope = np.asarray(results[c]["out_pe"], dtype=np.float64)[0]
        osr = np.asarray(results[c]["out_s"], dtype=np.float64)

        gsl = slice(GC * c, GC * (c + 1))
        U = U_all[gsl]
        D = D_all[gsl]
        stat = stat_all[gsl]

        SWON = -o[:, C_SWON]
        A_on = o[:, C_GUP] - U * o[:, C_SWON]   # = U*SWON + sum sw*(cs-csU)
        A_off = -o[:, C_GDN]
        SWT = -o[:, C_SWT0:C_SWT0 + 7]
        XON = -o[:, C_XON0:C_XON0 + 7]
        XOFF = -o[:, C_XOFF0:C_XOFF0 + 7]
        XTON = -o[:, C_XTON0:C_XTON0 + 7]
        XTOFF = -o[:, C_XTOFF0:C_XTOFF0 + 7]
        Praw = o[:, C_PFB0:C_PFB0 + 9]
        Pf = Praw - Praw[:, 0:1]

        mU = (tt_i < U[:, None])
        mD = (tt_i < D[:, None])
        up = A_on.sum()
        up -= (U[:, None] * SWT * mU).sum()
        up += (XTON * mU).sum()
        up -= (XON * mU).sum()
        dn = A_off.sum()
        dn -= (XTOFF * mD).sum()
        dn += (XOFF * mD).sum()
        rem_up = np.maximum(U - np.maximum(stat, 0), 0)
        rem_dn = np.maximum(D - np.maximum(-stat, 0), 0)
        g_idx = np.arange(GC)
        early = (B * rem_up - Pf[g_idx, rem_up]).sum() + Pf[g_idx, rem_dn].sum()
        viol += up + dn + early

        ed += (suc_all[gsl] * SWON).sum()
        bce_th += ob[:, 0].sum()

        ed += (segc_all[gsl].reshape(-1) * ope[0:2000]).sum()
        ed += (puc * ope[2000:2500]).sum()
        ed += (ccost * ope[2500:2900].reshape(S, BS).sum(axis=1)).sum()
        ed += (dcost * ope[2900:3300].reshape(S, BS).sum(axis=1)).sum()

        bce_ch += osr[:, 0].sum()
        bce_ds += osr[:, 1].sum()
        curt_sum += osr[:, 2].sum()

    n_th = float(B * G * T)
    n_s = float(B * S * T)
    sup = -(bce_th / n_th) - (bce_ch / n_s) - (bce_ds / n_s)
    total = (ed + POWER_BALANCE_PENALTY * curt_sum + sup
             + VIOLATIONS_PENALTY * viol)
    return np.float32(total)
